# revision 5
# baseline (speedup 1.0000x reference)
"""Trainium2 Bass kernel for nn_Block_13391708030014 (dense transformer block).

Sharding: data-parallel over batch — core b computes batch item b entirely
(B == n_cores == 8), no collectives.

Layout strategy per core:
  - layernorms run token-major ([128 tokens, C] tiles, bn_stats/bn_aggr),
    with the LN affine folded into the following matmul weights on the host;
  - activations are PE-transposed into feature-major ([C, tokens]) so every
    matmul contracts over the partition dim;
  - attention: logits are tiny (|s|<~1), so softmax skips max-subtraction:
    one Exp activation with accum_out gives both exp(s) and the row sum;
    the pos_2D blend is (1-a)/sum * e + a*pos, then the blended matrix is
    PE-transposed for the attn @ v matmul (v stays token-major);
  - the depthwise 3x3 conv runs as 9 accumulating diag-matmuls over a
    zero-padded 66x66 spatial layout, with bias+Gelu fused into the PSUM
    eviction;
  - fc2 accumulates into the residual tile in SBUF, which is DMA'd out.

All matmuls are bf16 (the residual stream stays fp32, so output error stays
in the few-1e-4 range).
"""

import numpy as np
import ml_dtypes

import concourse.bass as bass
import concourse.tile as tile
from concourse import mybir
from concourse.bass_utils import run_bass_kernel_spmd
from concourse.masks import make_identity

F32 = mybir.dt.float32
BF16 = mybir.dt.bfloat16
AF = mybir.ActivationFunctionType
OP = mybir.AluOpType

B, N, C = 8, 4096, 256
H, DH = 8, 32
NK = 256
HID = 1024
HW = 64
SR = 4
P = 128
TT = N // P          # 32 token tiles
KB = C // P          # 2 channel blocks
MB = HID // P        # 8 hidden blocks
PADW = HW + 2        # 66
NPAD = PADW * PADW   # 4356


def _split_drain_waits(nc, max_waits=1):
    """walrus in this toolchain refuses instructions with more than one sem
    wait; hoist every wait of a multi-wait instruction onto dedicated
    single-wait NOPs inserted just before it on the same engine (semantically
    identical: same engine, same program order)."""
    for f in nc.m.functions:
        for blk in f.blocks:
            insts = blk.instructions
            new = []
            changed = False
            for inst in insts:
                si = getattr(inst, "sync_info", None)
                if si is not None and si.on_wait and len(si.on_wait) > max_waits:
                    for i, w in enumerate(list(si.on_wait)):
                        new.append(mybir.InstNoOp(
                            name=f"{inst.name}-ws{i}",
                            sync_info=mybir.SyncInfo(on_wait=[w], on_update=[]),
                            bass_nofuse=True,
                            engine=inst.engine,
                        ))
                    inst.sync_info = mybir.SyncInfo(
                        on_wait=[], on_update=list(si.on_update or []))
                    changed = True
                new.append(inst)
            if changed:
                blk.instructions = new


def _bf(x):
    return np.ascontiguousarray(x.astype(ml_dtypes.bfloat16))


def _prep_weights(i):
    """Fold LN affines into downstream weights; return DRAM payloads."""
    ln1_w, ln1_b = i["ln1_w"], i["ln1_b"]
    srn_w, srn_b = i["srn_w"], i["srn_b"]
    ln2_w, ln2_b = i["ln2_w"], i["ln2_b"]

    qw = ln1_w[:, None] * i["q_w"]                      # [C, C]
    qb = ln1_b @ i["q_w"] + i["q_b"]                    # [C]

    # spatial-reduction conv as 16-tap matmul: srw[tap, ci, co]
    # sr_w is OIHW: [c_out, c_in, dy, dx] -> srw[tap, ci, co]
    srw = (i["sr_w"] * ln1_w[None, :, None, None]).transpose(2, 3, 1, 0)
    srw = np.ascontiguousarray(srw.reshape(SR * SR, C, C))
    srb = i["sr_b"] + np.einsum("i,oihw->o", ln1_b, i["sr_w"])

    kvw = srn_w[:, None] * i["kv_w"]                    # [C, 2C]
    kvb = srn_b @ i["kv_w"] + i["kv_b"]
    kw, vw = kvw[:, :C], kvw[:, C:]
    kb_, vb = kvb[:C], kvb[C:]

    f1w = ln2_w[:, None] * i["fc1_w"]                   # [C, HID]
    f1b = ln2_b @ i["fc1_w"] + i["fc1_b"]

    # depthwise conv -> block-diag matrices dwd[tap, mb, i, j]
    dww = i["dw_w"].reshape(HID, 9)                     # [HID, tap]
    dwd = np.zeros((9, MB, P, P), np.float32)
    idx = np.arange(P)
    for tap in range(9):
        for mb in range(MB):
            dwd[tap, mb, idx, idx] = dww[mb * P:(mb + 1) * P, tap]

    return {
        "qw": _bf(qw), "qb": qb.astype(np.float32),
        "srw": _bf(srw), "srb": srb.astype(np.float32),
        "kw": _bf(kw), "kb": kb_.astype(np.float32),
        "vw": _bf(vw), "vb": vb.astype(np.float32),
        "pjw": _bf(i["proj_w"]), "pjb": i["proj_b"].astype(np.float32),
        "f1w": _bf(f1w), "f1b": f1b.astype(np.float32),
        "dwd": _bf(dwd), "dwb": i["dw_b"].astype(np.float32),
        "f2w": _bf(i["fc2_w"]), "f2b": i["fc2_b"].astype(np.float32),
    }


def _build_program(a, nz):
    """Build the per-core Bass program. `a` = alpha scalar, `nz` = dict of
    which folded bias vectors are nonzero (adds emitted only when needed)."""
    nc = bass.Bass("TRN2", target_bir_lowering=False, debug=False,
                   num_devices=B)

    x_d = nc.dram_tensor("x", [N, C], F32, kind="ExternalInput").ap()
    pos_d = nc.dram_tensor("pos", [H, N, NK], F32, kind="ExternalInput").ap()
    out_d = nc.dram_tensor("out", [N, C], F32, kind="ExternalOutput").ap()

    w_d = {}
    wshapes = {
        "qw": ([C, C], BF16), "srw": ([16, C, C], BF16),
        "kw": ([C, C], BF16), "vw": ([C, C], BF16),
        "pjw": ([C, C], BF16), "f1w": ([C, HID], BF16),
        "dwd": ([9, MB, P, P], BF16), "dwb": ([HID], F32),
        "f2w": ([HID, C], BF16),
    }
    for nm in ("qb", "srb", "kb", "vb", "pjb", "f1b", "f2b"):
        if nz[nm]:
            wshapes[nm] = ([{"f1b": HID}.get(nm, C)], F32)
    for nm, (shp, dt) in wshapes.items():
        w_d[nm] = nc.dram_tensor(nm, shp, dt, kind="ExternalInput").ap()

    scale = DH ** -0.5

    with tile.TileContext(nc) as tc:
        from contextlib import ExitStack
        with ExitStack() as ctx:
            persist = ctx.enter_context(tc.tile_pool(name="persist", bufs=1))
            wpool = ctx.enter_context(tc.tile_pool(name="weights", bufs=1))
            stat = ctx.enter_context(tc.tile_pool(name="stat", bufs=6))

            # ---- persistent tiles
            hcT = [persist.tile([P, N], BF16, tag=f"hcT{k}", name=f"hcT{k}") for k in range(KB)]
            qT = [persist.tile([P, N], BF16, tag=f"qT{k}", name=f"qT{k}") for k in range(KB)]
            kT = [persist.tile([P, NK], BF16, tag=f"kT{k}", name=f"kT{k}") for k in range(KB)]
            vtok = [persist.tile([P, C], BF16, tag=f"vtok{k}", name=f"vtok{k}") for k in range(KB)]
            x2 = persist.tile([P, TT, C], F32, tag="x2")
            h2T = [persist.tile([P, N], BF16, tag=f"h2T{k}", name=f"h2T{k}") for k in range(KB)]

            # ---- weights to SBUF
            ident = wpool.tile([P, P], BF16)
            make_identity(nc, ident[:])
            eps1 = wpool.tile([P, 1], F32)
            nc.vector.memset(eps1[:], 1e-6)
            epss = wpool.tile([P, 1], F32)
            nc.vector.memset(epss[:], 1e-5)

            qw_sb = wpool.tile([P, KB, C], BF16)
            nc.sync.dma_start(qw_sb[:], w_d["qw"].rearrange("(k p) c -> p k c", p=P))
            srw_sb = wpool.tile([P, 16, KB, C], BF16)
            nc.sync.dma_start(srw_sb[:], w_d["srw"].rearrange("t (k p) c -> p t k c", p=P))
            kw_sb = wpool.tile([P, KB, C], BF16)
            nc.sync.dma_start(kw_sb[:], w_d["kw"].rearrange("(k p) c -> p k c", p=P))
            vw_sb = wpool.tile([P, KB, C], BF16)
            nc.sync.dma_start(vw_sb[:], w_d["vw"].rearrange("(k p) c -> p k c", p=P))
            pjw_sb = wpool.tile([P, KB, C], BF16)
            nc.sync.dma_start(pjw_sb[:], w_d["pjw"].rearrange("(k p) c -> p k c", p=P))
            f1w_sb = wpool.tile([P, KB, HID], BF16)
            nc.sync.dma_start(f1w_sb[:], w_d["f1w"].rearrange("(k p) c -> p k c", p=P))
            f2w_sb = wpool.tile([P, MB, C], BF16)
            nc.sync.dma_start(f2w_sb[:], w_d["f2w"].rearrange("(m p) c -> p m c", p=P))
            dwb_sb = wpool.tile([P, MB], F32)
            nc.sync.dma_start(dwb_sb[:], w_d["dwb"].rearrange("(m p) -> p m", p=P))

            bias_sb = {}
            for nm, dim in (("qb", C), ("srb", C), ("kb", C), ("f1b", HID)):
                if nz[nm]:
                    nb = dim // P
                    t = wpool.tile([P, nb], F32, name=f"bias_{nm}")
                    nc.sync.dma_start(t[:], w_d[nm].rearrange("(k p) -> p k", p=P))
                    bias_sb[nm] = t
            for nm in ("vb", "pjb", "f2b"):
                if nz[nm]:  # free-axis bias: broadcast across partitions
                    t = wpool.tile([P, C], F32, name=f"biasbc_{nm}")
                    nc.sync.dma_start(t[:], w_d[nm].to_broadcast([P, C]))
                    bias_sb[nm] = t

            # ================= phase A: ln1 + transpose =================
            with ExitStack() as pctx:
                xpool = pctx.enter_context(tc.tile_pool(name="xa", bufs=3))
                hcpool = pctx.enter_context(tc.tile_pool(name="hca", bufs=3))
                tp_ps = pctx.enter_context(
                    tc.tile_pool(name="tpA", bufs=4, space="PSUM"))
                for tt in range(TT):
                    xt = xpool.tile([P, C], F32)
                    nc.sync.dma_start(xt[:], x_d[tt * P:(tt + 1) * P, :])
                    st = stat.tile([P, 6], F32)
                    nc.vector.bn_stats(out=st[:], in_=xt[:])
                    mv = stat.tile([P, 2], F32)
                    nc.vector.bn_aggr(out=mv[:], in_=st[:])
                    rs = stat.tile([P, 1], F32)
                    nc.scalar.activation(rs[:], mv[:, 1:2], AF.Sqrt, bias=eps1[:])
                    nc.vector.reciprocal(rs[:], rs[:])
                    hc = hcpool.tile([P, C], BF16)
                    nc.vector.tensor_scalar(
                        out=hc[:], in0=xt[:], scalar1=mv[:, 0:1], scalar2=rs[:],
                        op0=OP.subtract, op1=OP.mult)
                    for kb in range(KB):
                        pt = tp_ps.tile([P, P], BF16)
                        nc.tensor.transpose(pt[:], hc[:, kb * P:(kb + 1) * P], ident[:])
                        nc.vector.tensor_copy(
                            out=hcT[kb][:, tt * P:(tt + 1) * P], in_=pt[:])

            # ================= phase B: q, SR-conv, srn, k, v ============
            with ExitStack() as pctx:
                mm_ps = pctx.enter_context(
                    tc.tile_pool(name="mmB", bufs=2, space="PSUM"))
                tp_ps = pctx.enter_context(
                    tc.tile_pool(name="tpB", bufs=4, space="PSUM"))
                bwork = pctx.enter_context(tc.tile_pool(name="bwork", bufs=1))

                # q^T
                for cb in range(KB):
                    for nt in range(8):
                        ps = mm_ps.tile([P, 512], F32, tag="mm")
                        for kb in range(KB):
                            nc.tensor.matmul(
                                ps[:], qw_sb[:, kb, cb * P:(cb + 1) * P],
                                hcT[kb][:, nt * 512:(nt + 1) * 512],
                                start=(kb == 0), stop=(kb == KB - 1))
                        dst = qT[cb][:, nt * 512:(nt + 1) * 512]
                        if nz["qb"]:
                            nc.vector.tensor_scalar(
                                out=dst, in0=ps[:], scalar1=bias_sb["qb"][:, cb:cb + 1],
                                scalar2=None, op0=OP.add)
                        else:
                            nc.vector.tensor_copy(out=dst, in_=ps[:])

                # SR conv -> hsT (feature-major [co, nk])
                hsT = [bwork.tile([P, NK], BF16, tag=f"hsT{c}", name=f"hsT{c}") for c in range(KB)]
                conv_rhs = [
                    hcT[kb].rearrange("p (r a c b) -> p a b r c", a=SR, b=SR, c=HW // SR)
                    for kb in range(KB)]
                for cob in range(KB):
                    ps = mm_ps.tile([P, NK], F32, tag="mm")
                    first = True
                    for tap in range(16):
                        dy, dx = tap // SR, tap % SR
                        for kb in range(KB):
                            nc.tensor.matmul(
                                ps[:], srw_sb[:, tap, kb, cob * P:(cob + 1) * P],
                                conv_rhs[kb][:, dy, dx, :, :],
                                start=first, stop=(tap == 15 and kb == KB - 1))
                            first = False
                    if nz["srb"]:
                        nc.vector.tensor_scalar(
                            out=hsT[cob][:], in0=ps[:],
                            scalar1=bias_sb["srb"][:, cob:cob + 1],
                            scalar2=None, op0=OP.add)
                    else:
                        nc.vector.tensor_copy(out=hsT[cob][:], in_=ps[:])

                # srn layernorm (transpose -> stats -> normalize -> transpose)
                hs_tok = [bwork.tile([P, C], BF16, tag=f"hstok{k}", name=f"hstok{k}") for k in range(KB)]
                for nkb in range(KB):
                    for cb in range(KB):
                        pt = tp_ps.tile([P, P], BF16)
                        nc.tensor.transpose(
                            pt[:], hsT[cb][:, nkb * P:(nkb + 1) * P], ident[:])
                        nc.vector.tensor_copy(
                            out=hs_tok[nkb][:, cb * P:(cb + 1) * P], in_=pt[:])
                hsnT = [bwork.tile([P, NK], BF16, tag=f"hsnT{k}", name=f"hsnT{k}") for k in range(KB)]
                for nkb in range(KB):
                    st = stat.tile([P, 6], F32)
                    nc.vector.bn_stats(out=st[:], in_=hs_tok[nkb][:])
                    mv = stat.tile([P, 2], F32)
                    nc.vector.bn_aggr(out=mv[:], in_=st[:])
                    rs = stat.tile([P, 1], F32)
                    nc.scalar.activation(rs[:], mv[:, 1:2], AF.Sqrt, bias=epss[:])
                    nc.vector.reciprocal(rs[:], rs[:])
                    hsn = bwork.tile([P, C], BF16, tag=f"hsn{nkb}")
                    nc.vector.tensor_scalar(
                        out=hsn[:], in0=hs_tok[nkb][:], scalar1=mv[:, 0:1],
                        scalar2=rs[:], op0=OP.subtract, op1=OP.mult)
                    for cb in range(KB):
                        pt = tp_ps.tile([P, P], BF16)
                        nc.tensor.transpose(
                            pt[:], hsn[:, cb * P:(cb + 1) * P], ident[:])
                        nc.vector.tensor_copy(
                            out=hsnT[cb][:, nkb * P:(nkb + 1) * P], in_=pt[:])

                # k^T [c, nk]
                for cb in range(KB):
                    ps = mm_ps.tile([P, NK], F32, tag="mm")
                    for kb in range(KB):
                        nc.tensor.matmul(
                            ps[:], kw_sb[:, kb, cb * P:(cb + 1) * P], hsnT[kb][:],
                            start=(kb == 0), stop=(kb == KB - 1))
                    if nz["kb"]:
                        nc.vector.tensor_scalar(
                            out=kT[cb][:], in0=ps[:],
                            scalar1=bias_sb["kb"][:, cb:cb + 1],
                            scalar2=None, op0=OP.add)
                    else:
                        nc.vector.tensor_copy(out=kT[cb][:], in_=ps[:])
                # v token-major [nk, c]
                for nkb in range(KB):
                    ps = mm_ps.tile([P, C], F32, tag="mm")
                    for kb in range(KB):
                        nc.tensor.matmul(
                            ps[:], hsnT[kb][:, nkb * P:(nkb + 1) * P],
                            vw_sb[:, kb, :],
                            start=(kb == 0), stop=(kb == KB - 1))
                    nc.vector.tensor_copy(out=vtok[nkb][:], in_=ps[:])
                    if nz["vb"]:
                        nc.vector.tensor_add(
                            out=vtok[nkb][:], in0=vtok[nkb][:], in1=bias_sb["vb"][:])

            # ================= phase C: attention ========================
            with ExitStack() as pctx:
                xpool = pctx.enter_context(tc.tile_pool(name="xc", bufs=3))
                pospool = pctx.enter_context(tc.tile_pool(name="pos", bufs=6))
                epool = pctx.enter_context(tc.tile_pool(name="eatt", bufs=3))
                bpool = pctx.enter_context(tc.tile_pool(name="batt", bufs=4))
                btp = pctx.enter_context(tc.tile_pool(name="btp", bufs=4))
                otpool = pctx.enter_context(tc.tile_pool(name="otp", bufs=2))
                s_ps = pctx.enter_context(
                    tc.tile_pool(name="sps", bufs=2, space="PSUM"))
                tp_ps = pctx.enter_context(
                    tc.tile_pool(name="tpC", bufs=2, space="PSUM"))
                o_ps = pctx.enter_context(
                    tc.tile_pool(name="ops", bufs=2, space="PSUM"))
                pj_ps = pctx.enter_context(
                    tc.tile_pool(name="pjps", bufs=2, space="PSUM"))

                for tt in range(TT):
                    xt = xpool.tile([P, C], F32)
                    nc.sync.dma_start(xt[:], x_d[tt * P:(tt + 1) * P, :])
                    oTs = otpool.tile([P, KB, P], BF16)
                    for hg in range(KB):
                        op_t = o_ps.tile([P, P], F32)
                        for hh in range(4):
                            h = hg * 4 + hh
                            post = pospool.tile([P, NK], F32)
                            nc.sync.dma_start(
                                post[:], pos_d[h, tt * P:(tt + 1) * P, :])
                            sps = s_ps.tile([P, NK], F32)
                            nc.tensor.matmul(
                                sps[:],
                                qT[hg][hh * 32:(hh + 1) * 32, tt * P:(tt + 1) * P],
                                kT[hg][hh * 32:(hh + 1) * 32, :],
                                start=True, stop=True,
                                tile_position=(hh * 32, 0))
                            e = epool.tile([P, NK], F32)
                            ssum = stat.tile([P, 1], F32)
                            nc.scalar.activation(
                                e[:], sps[:], AF.Exp, scale=scale,
                                accum_out=ssum[:])
                            f = stat.tile([P, 1], F32)
                            nc.vector.reciprocal(f[:], ssum[:])
                            es = bpool.tile([P, NK], BF16, tag="es")
                            nc.vector.tensor_scalar(
                                out=es[:], in0=e[:], scalar1=f[:],
                                scalar2=1.0 - a, op0=OP.mult, op1=OP.mult)
                            poss = bpool.tile([P, NK], BF16, tag="poss")
                            nc.gpsimd.tensor_scalar(
                                out=poss[:], in0=post[:], scalar1=a,
                                scalar2=None, op0=OP.mult)
                            bl = bpool.tile([P, NK], BF16, tag="bl")
                            nc.vector.tensor_add(out=bl[:], in0=es[:], in1=poss[:])
                            for nkb in range(KB):
                                pt = tp_ps.tile([P, P], BF16)
                                nc.tensor.transpose(
                                    pt[:], bl[:, nkb * P:(nkb + 1) * P], ident[:])
                                blT = btp.tile([P, P], BF16)
                                nc.scalar.copy(out=blT[:], in_=pt[:])
                                nc.tensor.matmul(
                                    op_t[hh * 32:(hh + 1) * 32, :],
                                    vtok[nkb][:, h * 32:(h + 1) * 32],
                                    blT[:],
                                    start=(nkb == 0), stop=(nkb == KB - 1),
                                    tile_position=(0, hh * 32))
                        nc.vector.tensor_copy(out=oTs[:, hg, :], in_=op_t[:])
                    pps = pj_ps.tile([P, C], F32)
                    for hg in range(KB):
                        nc.tensor.matmul(
                            pps[:], oTs[:, hg, :], pjw_sb[:, hg, :],
                            start=(hg == 0), stop=(hg == KB - 1))
                    if nz["pjb"]:
                        nc.vector.tensor_add(
                            out=x2[:, tt, :], in0=pps[:], in1=bias_sb["pjb"][:])
                        nc.vector.tensor_add(
                            out=x2[:, tt, :], in0=x2[:, tt, :], in1=xt[:])
                    else:
                        nc.vector.tensor_tensor(
                            out=x2[:, tt, :], in0=xt[:], in1=pps[:], op=OP.add)

            # ================= phase D: MLP ==============================
            with ExitStack() as pctx:
                h2pool = pctx.enter_context(tc.tile_pool(name="h2c", bufs=3))
                mpadp = pctx.enter_context(tc.tile_pool(name="mpad", bufs=2))
                m2cp = pctx.enter_context(tc.tile_pool(name="m2c", bufs=2))
                dwdp = pctx.enter_context(tc.tile_pool(name="dwd", bufs=2))
                mm_ps = pctx.enter_context(
                    tc.tile_pool(name="mmD", bufs=3, space="PSUM"))
                f2_ps = pctx.enter_context(
                    tc.tile_pool(name="f2ps", bufs=2, space="PSUM"))
                tp_ps = pctx.enter_context(
                    tc.tile_pool(name="tpD", bufs=2, space="PSUM"))

                # ln2 + transpose
                for tt in range(TT):
                    st = stat.tile([P, 6], F32)
                    nc.vector.bn_stats(out=st[:], in_=x2[:, tt, :])
                    mv = stat.tile([P, 2], F32)
                    nc.vector.bn_aggr(out=mv[:], in_=st[:])
                    rs = stat.tile([P, 1], F32)
                    nc.scalar.activation(rs[:], mv[:, 1:2], AF.Sqrt, bias=eps1[:])
                    nc.vector.reciprocal(rs[:], rs[:])
                    h2c = h2pool.tile([P, C], BF16)
                    nc.vector.tensor_scalar(
                        out=h2c[:], in0=x2[:, tt, :], scalar1=mv[:, 0:1],
                        scalar2=rs[:], op0=OP.subtract, op1=OP.mult)
                    for kb in range(KB):
                        pt = tp_ps.tile([P, P], BF16)
                        nc.tensor.transpose(
                            pt[:], h2c[:, kb * P:(kb + 1) * P], ident[:])
                        nc.vector.tensor_copy(
                            out=h2T[kb][:, tt * P:(tt + 1) * P], in_=pt[:])

                for mb in range(MB):
                    # fc1 -> padded layout
                    mpad = mpadp.tile([P, NPAD], BF16)
                    nc.vector.memset(mpad[:], 0.0)
                    vp = mpad.rearrange("p (r c) -> p r c", c=PADW)
                    for nt in range(8):
                        ps = mm_ps.tile([P, 512], F32, tag="mmd")
                        for kb in range(KB):
                            nc.tensor.matmul(
                                ps[:], f1w_sb[:, kb, mb * P:(mb + 1) * P],
                                h2T[kb][:, nt * 512:(nt + 1) * 512],
                                start=(kb == 0), stop=(kb == KB - 1))
                        dst = vp[:, 1 + 8 * nt:1 + 8 * nt + 8, 1:65]
                        src = ps.rearrange("p (r c) -> p r c", c=HW)
                        if nz["f1b"]:
                            nc.vector.tensor_scalar(
                                out=dst, in0=src,
                                scalar1=bias_sb["f1b"][:, mb:mb + 1],
                                scalar2=None, op0=OP.add)
                        else:
                            nc.vector.tensor_copy(out=dst, in_=src)
                    # depthwise conv + gelu
                    dwd_sb = dwdp.tile([P, 9, P], BF16)
                    nc.sync.dma_start(
                        dwd_sb[:],
                        w_d["dwd"][:, mb, :, :].rearrange("t q c -> q t c"))
                    m2c = m2cp.tile([P, N], BF16)
                    for rb in range(8):
                        dps = mm_ps.tile([P, 512], F32, tag="mmd")
                        for tap in range(9):
                            dy, dx = tap // 3, tap % 3
                            rhs = vp[:, 8 * rb + dy:8 * rb + dy + 8, dx:dx + HW]
                            nc.tensor.matmul(
                                dps[:], dwd_sb[:, tap, :], rhs,
                                start=(tap == 0), stop=(tap == 8))
                        nc.scalar.activation(
                            m2c[:, rb * 512:(rb + 1) * 512], dps[:], AF.Gelu,
                            bias=dwb_sb[:, mb:mb + 1])
                    # fc2 partial accumulated into x2
                    for tt in range(TT):
                        fps = f2_ps.tile([P, C], F32)
                        nc.tensor.matmul(
                            fps[:], m2c[:, tt * P:(tt + 1) * P],
                            f2w_sb[:, mb, :], start=True, stop=True)
                        nc.vector.tensor_tensor(
                            out=x2[:, tt, :], in0=x2[:, tt, :], in1=fps[:],
                            op=OP.add)

                if nz["f2b"]:
                    for tt in range(TT):
                        nc.vector.tensor_add(
                            out=x2[:, tt, :], in0=x2[:, tt, :],
                            in1=bias_sb["f2b"][:])

                for tt in range(TT):
                    nc.sync.dma_start(
                        out_d[tt * P:(tt + 1) * P, :], x2[:, tt, :])

    _split_drain_waits(nc)
    return nc


def _run(inputs, trace=False):
    w = _prep_weights(inputs)
    a = float(np.asarray(inputs["alpha"]).reshape(-1)[0])
    nz = {nm: bool(np.any(w[nm])) for nm in
          ("qb", "srb", "kb", "vb", "pjb", "f1b", "f2b")}
    nc = _build_program(a, nz)

    x = np.asarray(inputs["x"], np.float32)
    pos = np.asarray(inputs["pos_2D"], np.float32)
    shared = {k: v for k, v in w.items()
              if k in ("qw", "srw", "kw", "vw", "pjw", "f1w", "dwd", "dwb", "f2w")}
    for nm in ("qb", "srb", "kb", "vb", "pjb", "f1b", "f2b"):
        if nz[nm]:
            shared[nm] = w[nm]
    in_maps = [dict(shared, x=np.ascontiguousarray(x[b]),
                    pos=np.ascontiguousarray(pos[b])) for b in range(B)]
    res = run_bass_kernel_spmd(nc, in_maps, list(range(B)), trace=trace)
    out = np.stack([res.results[b]["out"] for b in range(B)]).astype(np.float32)
    return out, res


def kernel(**inputs) -> np.ndarray:
    out, _ = _run(inputs, trace=False)
    return out
